# revision 2
# baseline (speedup 1.0000x reference)
"""Trainium2 Bass kernel for nn_Attention_Encode (B=4, N=2048, DIM=1024, H=16, DH=64).

Sharding: 16 heads -> 8 cores x 2 heads (tensor parallel). Each core computes
  ztu_g = W_g @ ZT^T          (its 128 output channels = 2 heads)
  attention per (batch, head) with Q=K=V=ztu
  partial_out = ssa_g @ W_g   (row-sharded output projection)
Host sums the 8 partials (the all-reduce step of a row-sharded projection).

On-device layout is fully transposed ("scoresT" = [keys, queries]) so that
softmax needs no transposes: the AV matmul's stationary operand [V | ones]
produces both the numerator and the softmax denominator.

v2: the two heads' QK matmuls are K=64 and run CONCURRENTLY via PE row
tiling (head A on array rows 0:63, head B on rows 64:127) instead of being
zero-padded to K=128 and serialized.  ztuT holds head A d-dims on SBUF
partitions 0:63 and head B on 64:127, so tile_position is auto-derived
from the operands' base partitions.
"""
import sys

for _p in ('/opt/trn_rl_repo',):
    if _p not in sys.path:
        sys.path.insert(0, _p)

from contextlib import ExitStack

import numpy as np
import ml_dtypes

import concourse.bacc as bacc
import concourse.mybir as mybir
import concourse.tile as tile
from concourse.bass_utils import run_bass_kernel_spmd
from concourse.masks import make_identity

B, N, C = 4, 2048, 1024          # batch, seq, model dim
KP, DH, HPER = 128, 64, 2        # per-core channels, head dim, heads per core
NQB = 512                        # query block
NKT = 128                        # key tile
NTB = N // NKT                   # 16 key tiles per batch
NTILES = B * NTB                 # 64 n-tiles total
SCALE = DH ** -0.5               # 0.125
BF = mybir.dt.bfloat16
F32 = mybir.dt.float32
F32R = mybir.dt.float32r

_CACHE = {}


def _build_kernel():
    nc = bacc.Bacc("TRN2", target_bir_lowering=False, debug=False)
    ztt = nc.dram_tensor("ztt", [B, C, N], BF, kind="ExternalInput").ap()
    wgt = nc.dram_tensor("wgt", [C, KP], BF, kind="ExternalInput").ap()   # W_g^T
    wg = nc.dram_tensor("wg", [KP, C], BF, kind="ExternalInput").ap()     # W_g
    out = nc.dram_tensor("out", [B * N, C], BF, kind="ExternalOutput").ap()

    with tile.TileContext(nc) as tc, ExitStack() as ctx:
        _body(ctx, tc, ztt, wgt, wg, out)
    nc.compile()
    return nc


def _body(ctx, tc, ztt, wgt, wg, out):
    nc = tc.nc
    singles = ctx.enter_context(tc.tile_pool(name="singles", bufs=1))
    zin_pool = ctx.enter_context(tc.tile_pool(name="zin", bufs=16))
    sc_pool = ctx.enter_context(tc.tile_pool(name="sc", bufs=2, space="PSUM"))
    av_pool = ctx.enter_context(tc.tile_pool(name="av", bufs=2, space="PSUM"))
    p2_pool = ctx.enter_context(tc.tile_pool(name="p2", bufs=2, space="PSUM"))
    ex_pool = ctx.enter_context(tc.tile_pool(name="ex", bufs=12))
    sn_pool = ctx.enter_context(tc.tile_pool(name="sn", bufs=4))
    rc_pool = ctx.enter_context(tc.tile_pool(name="rc", bufs=4))

    # ---- persistent SBUF ----
    wgt_sb = singles.tile([128, 8, KP], BF)            # [c-in-tile, ci, k]
    nc.sync.dma_start(out=wgt_sb, in_=wgt.rearrange("(ci p) k -> p ci k", p=128))
    wg_sb = singles.tile([KP, C], BF)
    nc.sync.dma_start(out=wg_sb, in_=wg)
    ident = singles.tile([128, 128], BF)
    make_identity(nc, ident)
    self_f = singles.tile([128, 128], F32)
    nc.vector.memset(self_f, 0.0)
    nc.vector.memset(self_f[0:1, 0:64], 1.0)
    nc.vector.memset(self_f[32:33, 64:128], 1.0)
    sel = singles.tile([128, 128], F32R)               # den -> per-head row broadcast
    nc.vector.tensor_copy(out=sel, in_=self_f)
    dn = singles.tile([128, NQB], F32R)                # dens: head A row 0, head B row 32
    nc.vector.memset(dn[:].bitcast(F32), 0.0)
    # ztu^T packed: SBUF partitions 0:64 = head A d-dims, 64:128 = head B.
    # QK matmuls slice 64 rows per head and run concurrently via PE row tiling.
    ztuT = singles.tile([128, B * N], BF)
    # v-natural per head, padded to M=128: cols [v(64) | ones(1) | 0...]
    ztuN = singles.tile([128, NTILES, 2 * NKT], BF)    # [n-in-tile, nt, head*128+c]
    nc.gpsimd.memset(ztuN, 0.0)
    nc.gpsimd.memset(ztuN[:, :, DH:DH + 1], 1.0)
    nc.gpsimd.memset(ztuN[:, :, NKT + DH:NKT + DH + 1], 1.0)

    # ---- phase 1: proj1 (ztuT = W_g @ ZT^T) + phase 1.5: transposes (ztuN) ----
    def load_zin(b):
        zin = []
        for ci in range(8):
            z = zin_pool.tile([128, N], BF, tag="zin", name=f"zin{ci}")
            for jn in range(N // NQB):
                nc.sync.dma_start(
                    out=z[:, jn * NQB:(jn + 1) * NQB],
                    in_=ztt[b, ci * 128:(ci + 1) * 128, jn * NQB:(jn + 1) * NQB])
            zin.append(z)
        return zin

    def proj1_chunk(b, zin, jn):
        p1 = sc_pool.tile([128, 2 * NQB], F32, tag="sc")
        p1v = p1[:, 0:NQB]
        for ci in range(8):
            nc.tensor.matmul(
                p1v, lhsT=wgt_sb[:, ci, :],
                rhs=zin[ci][:, jn * NQB:(jn + 1) * NQB],
                start=(ci == 0), stop=(ci == 7),
            )
        nc.vector.tensor_copy(
            out=ztuT[:, b * N + jn * NQB: b * N + (jn + 1) * NQB],
            in_=p1v)

    def transpose_chunk(b, jn):
        # One transpose per n-tile: ztuT rows 0:64 / 64:128 are heads A / B,
        # so pt cols 0:64 / 64:128 are the per-head v-naturals.
        for ntl in range(4 * jn, 4 * jn + 4):
            nt = b * NTB + ntl
            pt = av_pool.tile([128, NQB], BF, tag="av", name="pt")
            nc.tensor.transpose(
                pt[:, 0:128],
                ztuT[:, nt * NKT:(nt + 1) * NKT],
                ident,
            )
            for hh in range(HPER):
                nc.vector.tensor_copy(
                    out=ztuN[:, nt, hh * NKT: hh * NKT + DH],
                    in_=pt[:, hh * DH: hh * DH + DH])

    # ---- phase 2 defs: attention + proj2, software-pipelined across q-blocks ----
    # Emit q-block j's QK/exp/AV before q-block j-1's normalize+proj2 so the
    # PE queue (in-order) never stalls on the DVE normalization chain.
    def attention_block(b, jq, filler=None):
        # Pipelined within the q-block: group g+1's QK is emitted BEFORE
        # group g's AV, so the in-order PE queue never waits on exp(g) (ACT).
        # The final AV group is emitted after the filler for the same reason.
        q0 = b * N + jq * NQB
        avs = [av_pool.tile([128, NQB], F32, tag="av", name=f"av{h}")
               for h in range(HPER)]

        def emit_avs(g, exs):
            for hh in range(HPER):
                for u in range(2):
                    ik = 2 * g + u
                    vT = ztuN[:, b * NTB + ik, hh * NKT:(hh + 1) * NKT]
                    nc.tensor.matmul(avs[hh], lhsT=vT,
                                     rhs=exs[hh][:, u * NQB:(u + 1) * NQB],
                                     start=(ik == 0), stop=(ik == NTB - 1))

        prev = None
        for g in range(NTB // 2):               # groups of 2 key tiles
            scs, exs = [], []
            for hh in range(HPER):
                sc = sc_pool.tile([128, 2 * NQB], F32, tag="sc")
                h0 = hh * DH
                qT = ztuT[h0:h0 + DH, q0:q0 + NQB]
                for u in range(2):
                    ik = 2 * g + u
                    kT = ztuT[h0:h0 + DH,
                              b * N + ik * NKT: b * N + (ik + 1) * NKT]
                    nc.tensor.matmul(sc[:, u * NQB:(u + 1) * NQB],
                                     lhsT=kT, rhs=qT, start=True, stop=True)
                scs.append(sc)
            for hh in range(HPER):
                ex = ex_pool.tile([128, 2 * NQB], BF, tag="ex")
                nc.scalar.activation(
                    out=ex, in_=scs[hh],
                    func=mybir.ActivationFunctionType.Exp, scale=SCALE)
                exs.append(ex)
            if prev is not None:
                emit_avs(*prev)
            prev = (g, exs)
        if filler is not None:
            filler()
        emit_avs(*prev)
        return avs

    def finish_norm(b, jq, avs):
        # softmax denominators -> per-head broadcast -> reciprocal -> scale
        nc.vector.tensor_copy(out=dn[0:1, :], in_=avs[0][DH:DH + 1, :])
        nc.vector.tensor_copy(out=dn[32:33, :], in_=avs[1][DH:DH + 1, :])
        bc = p2_pool.tile([128, NQB], F32, tag="p2", name="bc")
        bcv = bc[:, 0:NQB]
        nc.tensor.matmul(bcv, lhsT=sel, rhs=dn, start=True, stop=True)
        rc = rc_pool.tile([128, NQB], F32)
        nc.vector.reciprocal_approx_fast(out=rc, in_=bcv)
        sn = sn_pool.tile([128, NQB], BF)
        nc.vector.tensor_tensor(
            out=sn[0:64, :], in0=avs[0][0:DH, :], in1=rc[0:64, :],
            op=mybir.AluOpType.mult)
        nc.vector.tensor_tensor(
            out=sn[64:128, :], in0=avs[1][0:DH, :], in1=rc[64:128, :],
            op=mybir.AluOpType.mult)
        return sn

    def finish_proj2(b, jq, sn):
        # proj2: out[q, :] += ssa_norm_g @ W_g  (both heads contracted)
        for t in range(NQB // 128):
            for ch in range(2):
                p2 = p2_pool.tile([128, NQB], F32, tag="p2", name="p2")
                p2v = p2[:, 0:512]
                nc.tensor.matmul(
                    p2v, lhsT=sn[:, t * 128:(t + 1) * 128],
                    rhs=wg_sb[:, ch * 512:(ch + 1) * 512],
                    start=True, stop=True)
                p2s = rc_pool.tile([128, 512], BF, tag="p2s")
                nc.vector.tensor_copy(out=p2s, in_=p2v)
                r0 = b * N + jq * NQB + t * 128
                nc.gpsimd.dma_start(
                    out=out[r0:r0 + 128, ch * 512:(ch + 1) * 512], in_=p2s)

    # ---- main schedule: batch b's proj1/transposes are interleaved into
    # batch b-1's attention at q-block granularity. The previous q-block's
    # norm chain is emitted BEFORE this q-block's QK so its DVE work (which
    # releases the av psum slots) is already done when the AVs need them. ----
    state = {"pending": None, "sn": None}

    def flush_norm():
        if state["pending"] is not None:
            state["sn"] = (state["pending"][0], state["pending"][1],
                           finish_norm(*state["pending"]))
            state["pending"] = None

    def flush_proj2():
        if state["sn"] is not None:
            finish_proj2(*state["sn"])
            state["sn"] = None

    def attention_batch(b, filler=None):
        for jq in range(N // NQB):
            flush_norm()
            fl = (lambda jq=jq: filler(jq)) if filler is not None else None
            avs = attention_block(b, jq, fl)
            flush_proj2()
            state["pending"] = (b, jq, avs)

    # PE warm-up spin: ~6us of dependency-free matmuls so the HAM clock gate
    # is already at 8/8 when the first DMA-gated proj1 matmul lands.
    warm = p2_pool.tile([128, NQB], F32, tag="p2", name="warm")
    for _ in range(256):
        nc.tensor.matmul(warm[:, 0:32], lhsT=ident, rhs=ident[:, 0:32],
                         start=True, stop=True)
    del warm

    zs = {0: load_zin(0)}
    for b in range(B):
        if b + 1 < B:
            zs[b + 1] = load_zin(b + 1)
        if b == 0:
            for jn in range(N // NQB):
                proj1_chunk(0, zs[0], jn)
                transpose_chunk(0, jn)
            zs.pop(0)
        else:
            zin = zs.pop(b)

            def filler(jq, b=b, zin=zin):
                proj1_chunk(b, zin, jq)
                transpose_chunk(b, jq)

            attention_batch(b - 1, filler)
    attention_batch(B - 1)
    flush_norm()
    flush_proj2()


def _get_nc():
    if "nc" not in _CACHE:
        _CACHE["nc"] = _build_kernel()
    return _CACHE["nc"]


def make_in_maps(ZT, W):
    ZT = np.asarray(ZT, dtype=np.float32)
    W = np.asarray(W, dtype=np.float32)
    ztt = np.ascontiguousarray(ZT.transpose(0, 2, 1)).astype(ml_dtypes.bfloat16)
    in_maps = []
    for c in range(8):
        wgf = W[c * KP:(c + 1) * KP, :]
        in_maps.append({
            "ztt": ztt,
            "wgt": np.ascontiguousarray(wgf.T).astype(ml_dtypes.bfloat16),
            "wg": np.ascontiguousarray(wgf).astype(ml_dtypes.bfloat16),
        })
    return in_maps


def kernel(ZT: np.ndarray, W: np.ndarray) -> np.ndarray:
    nc = _get_nc()
    res = run_bass_kernel_spmd(nc, make_in_maps(ZT, W), core_ids=list(range(8)))
    acc = np.zeros((B * N, C), dtype=np.float32)
    for r in res.results:
        acc += r["out"].astype(np.float32)
    return acc.reshape(B, N, C)


if __name__ == "__main__":
    rng = np.random.default_rng(0)
    zt = rng.standard_normal((B, N, C), dtype=np.float32)
    w = rng.standard_normal((KP * 8, C), dtype=np.float32) * C ** -0.5
    o = kernel(zt, w)
    print("out", o.shape, o.dtype, float(np.abs(o).mean()))
